# revision 2
# baseline (speedup 1.0000x reference)
"""LSTM-with-reset critic kernel for Trainium2 (8 NeuronCores) — v2.

Same episode-rebatching strategy as v1 (see kernel.py docstring): the
reset mask makes episodes independent; sort episodes by length, scan
over episode-step s with all active episodes batched on the free axis,
finish the rare >S_CUT episodes on host.

v2 device-side changes:
  * 3 input tensors (xg, wpack, bias) instead of 8, 1 packed f32 output
    instead of 3 — cuts per-call dispatch overhead.
  * x gathered as [128, 4, ntot] so each chunk's 4 d-tiles load in ONE
    DMA instead of 4.
  * all weights packed in one [128, 6G+2] bf16 tensor, loaded in 1 DMA.
  * ACT ops reordered sigmoid-first (i,f,o then tanh g / tanh c) to
    minimize activation-table switches per chunk.
  * merged-path / bias2 / sel2 machinery dropped (wide path everywhere).

Layouts (per core):
  xg    [128, 4, NTOT] bf16   gathered x; xg[p, d, col] = x_row[col][d*128+p]
  wpack [128, 6G+2]    bf16   [wihT d-tiles | whhT k-tiles | wprojT]
  bias  [128, 8]       f32    (b_ih+b_hh) column per gate-tile
  out   [1, ntot+4*128*E] f32 y ++ h-export (2 halves) ++ c-export
"""

import numpy as np
import ml_dtypes

T, B, D, H = 4096, 32, 512, 256
G = 4 * H  # 1024 gate rows
N_CORES = 8
BL = B // N_CORES  # lanes per core
CHUNK = 512  # free-dim chunk (= one PSUM bank of f32)
PAD = 16  # pad per-step episode counts to a multiple of this
S_CUT = 8  # device runs scan steps < S_CUT; longer episodes finish on host

_BF16 = ml_dtypes.bfloat16


def _episodes_per_core(reset: np.ndarray):
    """Per core: list of (lane, start, length) sorted by length desc."""
    eps_per_core = []
    for c in range(N_CORES):
        eps = []
        for lane in range(c * BL, (c + 1) * BL):
            r = reset[:, lane]
            starts = np.flatnonzero(r == 1)
            if len(starts) == 0 or starts[0] != 0:
                starts = np.concatenate([[0], starts])
            ends = np.concatenate([starts[1:], [T]])
            for s0, e0 in zip(starts.tolist(), ends.tolist()):
                eps.append((lane, s0, e0 - s0))
        eps.sort(key=lambda e: -e[2])
        eps_per_core.append(eps)
    return eps_per_core


def _schedule(eps_per_core):
    """Common (max-over-cores) padded step schedule -> (npad, offs, ntot)."""
    lmax = max(e[2] for eps in eps_per_core for e in eps)
    npad = []
    for s in range(lmax):
        n = max(sum(1 for e in eps if e[2] > s) for eps in eps_per_core)
        npad.append(-(-n // PAD) * PAD)
    offs = np.concatenate([[0], np.cumsum(npad)]).astype(np.int64)
    return npad, offs, int(offs[-1])


def _build_gather(eps, npad, offs, ntot):
    """Row indices into flat x [T*B] for one core; -1 marks padding."""
    gidx = np.full(ntot, -1, dtype=np.int64)
    for s in range(len(npad)):
        base = int(offs[s])
        rank = 0
        for lane, start, ln in eps:
            if ln <= s:
                break  # sorted desc: no more active episodes
            gidx[base + rank] = (start + s) * B + lane
            rank += 1
    return gidx


def _chunks(n, first_small=False):
    """Split n into chunks <= CHUNK; optionally ramp the first chunks up
    (128, 128, 256, ...) so the first matmuls start sooner after DMA."""
    sizes = []
    c0 = 0
    if first_small and n > 1024:
        for c in (128, 128, 256):
            sizes.append((c0, c))
            c0 += c
    while c0 < n:
        c = min(CHUNK, n - c0)
        sizes.append((c0, c))
        c0 += c
    return sizes


def _build_bass(npad, offs, ntot, export_n=0):
    import concourse.bacc as bacc
    import concourse.mybir as mybir
    import concourse.tile as tile

    f32 = mybir.dt.float32
    bf16 = mybir.dt.bfloat16
    SIG = mybir.ActivationFunctionType.Sigmoid
    TANH = mybir.ActivationFunctionType.Tanh

    nc = bacc.Bacc("TRN2", target_bir_lowering=False, debug=False,
                   num_devices=N_CORES)
    WCOLS = 6 * G + 2
    NOUT = ntot + 4 * 128 * export_n
    xg_d = nc.dram_tensor("xg", [128, 4, ntot], bf16, kind="ExternalInput").ap()
    wp_d = nc.dram_tensor("wpack", [128, WCOLS], bf16,
                          kind="ExternalInput").ap()
    bias_d = nc.dram_tensor("bias", [128, 8], f32, kind="ExternalInput").ap()
    out_d = nc.dram_tensor("out", [1, NOUT], f32, kind="ExternalOutput").ap()

    n0 = npad[0]
    lmax = len(npad)

    with tile.TileContext(nc) as tc:
        with (
            tc.tile_pool(name="weights", bufs=1) as wpool,
            tc.tile_pool(name="state", bufs=1) as spool,
            tc.tile_pool(name="xs", bufs=3) as xpool,
            tc.tile_pool(name="gates", bufs=3) as gpool,
            tc.tile_pool(name="psum", bufs=6, space="PSUM") as ppool,
        ):
            wp = wpool.tile([128, WCOLS], bf16, tag="wp", name="wp")
            nc.sync.dma_start(wp[:], wp_d[:])
            bias = wpool.tile([128, 8], f32, tag="bias", name="bias")
            nc.sync.dma_start(bias[:], bias_d[:])

            def wih(d, gt):
                c0 = d * G + gt * 128
                return wp[:, c0:c0 + 128]

            def whh(k, gt):
                c0 = 4 * G + k * G + gt * 128
                return wp[:, c0:c0 + 128]

            def wproj(k):
                return wp[:, 6 * G + k:6 * G + k + 1]

            # persistent state: h history (bf16, feeds matmuls) + c (f32)
            hh = [spool.tile([128, ntot], bf16, tag=f"hh{k}", name=f"hh{k}")
                  for k in range(2)]
            cc = [spool.tile([128, n0], f32, tag=f"cc{k}", name=f"cc{k}")
                  for k in range(2)]

            for s in range(lmax):
                off = int(offs[s])
                poff = int(offs[s - 1]) if s > 0 else 0
                for c0, C in _chunks(npad[s], first_small=(s == 0)):
                    xt = xpool.tile([128, 4, C], bf16, tag="x", name="x")
                    nc.sync.dma_start(xt[:], xg_d[:, :, off + c0:off + c0 + C])
                    for half in range(2):
                        ps = {}
                        for gi, gname in enumerate("ifgo"):
                            if s == 0 and gname == "f":
                                continue
                            gt = gi * 2 + half
                            p = ppool.tile([128, C], f32, tag="ps", name="ps",
                                           bufs=6)
                            for d in range(4):
                                nc.tensor.matmul(
                                    p[:], lhsT=wih(d, gt), rhs=xt[:, d, :],
                                    start=(d == 0), stop=(s == 0 and d == 3))
                            if s > 0:
                                for k in range(2):
                                    nc.tensor.matmul(
                                        p[:], lhsT=whh(k, gt),
                                        rhs=hh[k][:, poff + c0:poff + c0 + C],
                                        start=False, stop=(k == 1))
                            ps[gname] = p

                        def bcol(gi):
                            gt = gi * 2 + half
                            return bias[:, gt:gt + 1]

                        c_sl = cc[half][:, c0:c0 + C]
                        # sigmoids first, then tanhs: fewer ACT table switches
                        sig_i = gpool.tile([128, C], f32, tag="si", name="si")
                        nc.scalar.activation(sig_i[:], ps["i"][:], SIG,
                                             bias=bcol(0))
                        if s > 0:
                            sig_f = gpool.tile([128, C], f32, tag="sf",
                                               name="sf")
                            nc.scalar.activation(sig_f[:], ps["f"][:], SIG,
                                                 bias=bcol(1))
                        sig_o = gpool.tile([128, C], f32, tag="so", name="so")
                        nc.scalar.activation(sig_o[:], ps["o"][:], SIG,
                                             bias=bcol(3))
                        tanh_g = gpool.tile([128, C], f32, tag="tg", name="tg")
                        nc.scalar.activation(tanh_g[:], ps["g"][:], TANH,
                                             bias=bcol(2))
                        if s == 0:
                            nc.vector.tensor_mul(c_sl, sig_i[:], tanh_g[:])
                        else:
                            nc.vector.tensor_mul(c_sl, c_sl, sig_f[:])
                            t1 = gpool.tile([128, C], f32, tag="t1", name="t1")
                            nc.vector.tensor_mul(t1[:], sig_i[:], tanh_g[:])
                            nc.vector.tensor_add(c_sl, c_sl, t1[:])
                        tanh_c = gpool.tile([128, C], f32, tag="tc", name="tc")
                        nc.scalar.activation(tanh_c[:], c_sl, TANH)
                        nc.vector.tensor_mul(hh[half][:, off + c0:off + c0 + C],
                                             sig_o[:], tanh_c[:])

            if export_n:
                lo = int(offs[lmax - 1])
                E = export_n
                for k in range(2):
                    hst = gpool.tile([128, E], f32, tag="hst", name="hst")
                    nc.vector.tensor_copy(hst[:], hh[k][:, lo:lo + E])
                    nc.sync.dma_start(
                        out_d[:, ntot + k * 128 * E:ntot + (k + 1) * 128 * E],
                        hst[:])
                    nc.sync.dma_start(
                        out_d[:, ntot + (2 + k) * 128 * E:
                              ntot + (3 + k) * 128 * E],
                        cc[k][:, 0:E])

            # projection pass: y = W_proj @ h  (b_proj added on host)
            for c0, C in _chunks(ntot):
                p = ppool.tile([1, C], f32, tag="psy", name="psy", bufs=2)
                for k in range(2):
                    nc.tensor.matmul(p[:], lhsT=wproj(k),
                                     rhs=hh[k][:, c0:c0 + C],
                                     start=(k == 0), stop=(k == 1))
                ysb = gpool.tile([1, C], f32, tag="ysb", name="ysb")
                nc.vector.tensor_copy(ysb[:], p[:])
                nc.sync.dma_start(out_d[:, c0:c0 + C], ysb[:])

    nc.compile()
    return nc


def _prep(inputs, eps_per_core, npad, offs, ntot):
    """Build (nc, in_maps) for the SPMD run. npad/offs/ntot are the
    device-side (possibly S_CUT-truncated) schedule."""
    x = np.asarray(inputs["x"], dtype=np.float32)

    # wpack: [wihT (4 d-tiles) | whhT (2 k-tiles) | wprojT (2 cols)] bf16
    wih_t = np.asarray(inputs["W_ih"], np.float32).T  # [D, 4H] = [512, G]
    whh_t = np.asarray(inputs["W_hh"], np.float32).T  # [H, G] = [256, G]
    wproj_t = np.asarray(inputs["W_proj"], np.float32).T  # [H, 1]
    WCOLS = 6 * G + 2
    wpack = np.zeros((128, WCOLS), dtype=np.float32)
    for d in range(4):
        wpack[:, d * G:(d + 1) * G] = wih_t[d * 128:(d + 1) * 128, :]
    for k in range(2):
        wpack[:, 4 * G + k * G:4 * G + (k + 1) * G] = \
            whh_t[k * 128:(k + 1) * 128, :]
    for k in range(2):
        wpack[:, 6 * G + k] = wproj_t[k * 128:(k + 1) * 128, 0]
    wpack = wpack.astype(_BF16)

    bias_flat = (np.asarray(inputs["b_ih"], np.float32)
                 + np.asarray(inputs["b_hh"], np.float32))
    bias_r = np.ascontiguousarray(bias_flat.reshape(8, 128).T)

    x2d = x.reshape(T * B, D)
    in_maps = []
    for c in range(N_CORES):
        gidx = _build_gather(eps_per_core[c], npad, offs, ntot)
        xr = x2d[np.maximum(gidx, 0)]       # [NTOT, D] f32
        xr[gidx < 0] = 0.0
        # xg[p, d, col] = xr[col, d*128+p]
        xg = np.ascontiguousarray(
            xr.T.reshape(4, 128, ntot).transpose(1, 0, 2)).astype(_BF16)
        in_maps.append({"xg": xg, "wpack": wpack, "bias": bias_r})

    export_n = npad[-1] if len(npad) == S_CUT else 0
    nc = _build_bass(npad, offs, ntot, export_n=export_n)
    return nc, in_maps


def _host_tail(out, hexp_cexp, eps_per_core, npad, inputs):
    """Finish episodes longer than S_CUT in exact f32 on the host, starting
    from the device-exported (h, c) state at step S_CUT-1."""
    x2d = np.asarray(inputs["x"], np.float32).reshape(T * B, D)
    W_ih = np.asarray(inputs["W_ih"], np.float32)
    W_hh = np.asarray(inputs["W_hh"], np.float32)
    bvec = (np.asarray(inputs["b_ih"], np.float32)
            + np.asarray(inputs["b_hh"], np.float32))
    W_proj = np.asarray(inputs["W_proj"], np.float32).reshape(-1)
    bp = np.float32(np.asarray(inputs["b_proj"]).reshape(-1)[0])
    s_cut = len(npad)
    sig = lambda v: 1.0 / (1.0 + np.exp(-v))
    for c in range(N_CORES):
        eps = [e for e in eps_per_core[c] if e[2] > s_cut]
        if not eps:
            continue
        n = len(eps)  # eps are ranks 0..n-1 (sorted desc, stable)
        hexp, cexp = hexp_cexp[c]
        h = hexp.reshape(256, -1)[:, :n].T.copy()  # [n, 256]
        cst = cexp.reshape(256, -1)[:, :n].T.copy()
        alive = list(range(n))
        s = s_cut
        while alive:
            keep = [i for i in alive if eps[i][2] > s]
            if not keep:
                break
            rows = np.array([(eps[i][1] + s) * B + eps[i][0] for i in keep])
            idx = np.array(keep)
            g = x2d[rows] @ W_ih.T + h[idx] @ W_hh.T + bvec
            i_, f_, g_, o_ = np.split(g, 4, axis=1)
            cst[idx] = sig(f_) * cst[idx] + sig(i_) * np.tanh(g_)
            hn = sig(o_) * np.tanh(cst[idx])
            h[idx] = hn
            out[rows] = hn @ W_proj + bp
            alive = keep
            s += 1


def _device_schedule(eps_per_core):
    """Full schedule truncated to the device's S_CUT window."""
    npad, offs, ntot = _schedule(eps_per_core)
    if len(npad) > S_CUT:
        npad = npad[:S_CUT]
        offs = offs[:S_CUT + 1]
        ntot = int(offs[-1])
    return npad, offs, ntot


def kernel(x, reset, W_ih, W_hh, b_ih, b_hh, W_proj, b_proj):
    from concourse.bass_utils import run_bass_kernel_spmd

    inputs = dict(x=x, reset=reset, W_ih=W_ih, W_hh=W_hh, b_ih=b_ih,
                  b_hh=b_hh, W_proj=W_proj, b_proj=b_proj)
    reset = np.asarray(reset)
    eps_per_core = _episodes_per_core(reset)
    npad, offs, ntot = _device_schedule(eps_per_core)
    nc, in_maps = _prep(inputs, eps_per_core, npad, offs, ntot)
    res = run_bass_kernel_spmd(nc, in_maps, core_ids=list(range(N_CORES)))

    export_n = npad[-1] if len(npad) == S_CUT else 0
    out = np.empty(T * B, dtype=np.float32)
    bp = np.float32(np.asarray(b_proj).reshape(-1)[0])
    hexp_cexp = []
    for c in range(N_CORES):
        gidx = _build_gather(eps_per_core[c], npad, offs, ntot)
        full = np.asarray(res.results[c]["out"]).reshape(-1)
        y = full[:ntot]
        valid = gidx >= 0
        out[gidx[valid]] = y[valid] + bp
        if export_n:
            E = export_n
            hexp = full[ntot:ntot + 2 * 128 * E].reshape(2, 128, E)
            cexp = full[ntot + 2 * 128 * E:ntot + 4 * 128 * E].reshape(
                2, 128, E)
            hexp_cexp.append((
                np.concatenate([hexp[0], hexp[1]], axis=0),
                np.concatenate([cexp[0], cexp[1]], axis=0)))
    if export_n:
        _host_tail(out, hexp_cexp, eps_per_core, npad, inputs)
    return out.reshape(T, B, 1)


# revision 3
# speedup vs baseline: 1.0578x; 1.0578x over previous
"""LSTM-with-reset critic kernel for Trainium2 (8 NeuronCores).

Strategy
--------
The reset mask zeroes (h, c) at episode starts, so each batch lane's
timeline splits into independent episodes (~2 steps mean with
Bernoulli(0.5) resets). Instead of a T=4096 sequential scan we:

1. (host) split every lane's timeline into episodes, sort by length
   (desc), and build a step-major gather of x: for scan step s, the rows
   of all episodes whose length > s form a contiguous block.
2. (device, data-parallel over B: 4 lanes/core) scan s = 0..S_CUT-1;
   each step is a large batched matmul whose free-axis width (number of
   still-active episodes) roughly halves every step. Everything stays
   feature-major (features on SBUF partitions, episodes on the free
   axis) so the recurrence needs no transposes.
3. (device) project h -> y with a final matmul pass; (host) finish the
   rare episodes longer than S_CUT from device-exported (h, c) state,
   and scatter y back to [T, B, 1].

Device-side layout choices:
  * 3 input tensors (xg, wpack, bias) instead of 8, 1 packed f32 output
    instead of 3 — cuts per-call dispatch overhead.
  * x gathered as [128, 4, ntot] so each chunk's 4 d-tiles load in ONE
    DMA instead of 4.
  * all weights packed in one [128, 6G+2] bf16 tensor, loaded in 1 DMA.
  * ACT ops reordered sigmoid-first (i,f,o then tanh g / tanh c) to
    minimize activation-table switches per chunk.
  * merged-path / bias2 / sel2 machinery dropped (wide path everywhere).

Layouts (per core):
  xg    [128, 4, NTOT] bf16   gathered x; xg[p, d, col] = x_row[col][d*128+p]
  wpack [128, 6G+2]    bf16   [wihT d-tiles | whhT k-tiles | wprojT]
  bias  [128, 8]       f32    (b_ih+b_hh) column per gate-tile
  out   [1, ntot+4*128*E] f32 y ++ h-export (2 halves) ++ c-export
"""

import numpy as np
import ml_dtypes

T, B, D, H = 4096, 32, 512, 256
G = 4 * H  # 1024 gate rows
N_CORES = 8
BL = B // N_CORES  # lanes per core
CHUNK = 512  # free-dim chunk (= one PSUM bank of f32)
PAD = 16  # pad per-step episode counts to a multiple of this
S_CUT = 8  # device runs scan steps < S_CUT; longer episodes finish on host

_BF16 = ml_dtypes.bfloat16


def _episodes_per_core(reset: np.ndarray):
    """Per core: list of (lane, start, length) sorted by length desc."""
    eps_per_core = []
    for c in range(N_CORES):
        eps = []
        for lane in range(c * BL, (c + 1) * BL):
            r = reset[:, lane]
            starts = np.flatnonzero(r == 1)
            if len(starts) == 0 or starts[0] != 0:
                starts = np.concatenate([[0], starts])
            ends = np.concatenate([starts[1:], [T]])
            for s0, e0 in zip(starts.tolist(), ends.tolist()):
                eps.append((lane, s0, e0 - s0))
        eps.sort(key=lambda e: -e[2])
        eps_per_core.append(eps)
    return eps_per_core


def _schedule(eps_per_core):
    """Common (max-over-cores) padded step schedule -> (npad, offs, ntot)."""
    lmax = max(e[2] for eps in eps_per_core for e in eps)
    npad = []
    for s in range(lmax):
        n = max(sum(1 for e in eps if e[2] > s) for eps in eps_per_core)
        npad.append(-(-n // PAD) * PAD)
    offs = np.concatenate([[0], np.cumsum(npad)]).astype(np.int64)
    return npad, offs, int(offs[-1])


def _build_gather(eps, npad, offs, ntot):
    """Row indices into flat x [T*B] for one core; -1 marks padding."""
    gidx = np.full(ntot, -1, dtype=np.int64)
    for s in range(len(npad)):
        base = int(offs[s])
        rank = 0
        for lane, start, ln in eps:
            if ln <= s:
                break  # sorted desc: no more active episodes
            gidx[base + rank] = (start + s) * B + lane
            rank += 1
    return gidx


def _chunks(n, first_small=False):
    """Split n into chunks <= CHUNK; optionally ramp the first chunks up
    (128, 128, 256, ...) so the first matmuls start sooner after DMA."""
    sizes = []
    c0 = 0
    if first_small and n > 1024:
        for c in (128, 128, 256):
            sizes.append((c0, c))
            c0 += c
    while c0 < n:
        c = min(CHUNK, n - c0)
        sizes.append((c0, c))
        c0 += c
    return sizes


def _build_bass(npad, offs, ntot, export_n=0):
    import concourse.bacc as bacc
    import concourse.mybir as mybir
    import concourse.tile as tile

    f32 = mybir.dt.float32
    bf16 = mybir.dt.bfloat16
    SIG = mybir.ActivationFunctionType.Sigmoid
    TANH = mybir.ActivationFunctionType.Tanh

    nc = bacc.Bacc("TRN2", target_bir_lowering=False, debug=False,
                   num_devices=N_CORES)
    WCOLS = 6 * G + 2
    NOUT = ntot + 4 * 128 * export_n
    xg_d = nc.dram_tensor("xg", [128, 4, ntot], bf16, kind="ExternalInput").ap()
    wp_d = nc.dram_tensor("wpack", [128, WCOLS], bf16,
                          kind="ExternalInput").ap()
    bias_d = nc.dram_tensor("bias", [128, 8], f32, kind="ExternalInput").ap()
    out_d = nc.dram_tensor("out", [1, NOUT], f32, kind="ExternalOutput").ap()

    n0 = npad[0]
    lmax = len(npad)

    with tile.TileContext(nc) as tc:
        with (
            tc.tile_pool(name="weights", bufs=1) as wpool,
            tc.tile_pool(name="state", bufs=1) as spool,
            tc.tile_pool(name="xs", bufs=3) as xpool,
            tc.tile_pool(name="gates", bufs=3) as gpool,
            tc.tile_pool(name="psum", bufs=6, space="PSUM") as ppool,
        ):
            wp = wpool.tile([128, WCOLS], bf16, tag="wp", name="wp")
            nc.sync.dma_start(wp[:], wp_d[:])
            bias = wpool.tile([128, 8], f32, tag="bias", name="bias")
            nc.sync.dma_start(bias[:], bias_d[:])

            def wih(d, gt):
                c0 = d * G + gt * 128
                return wp[:, c0:c0 + 128]

            def whh(k, gt):
                c0 = 4 * G + k * G + gt * 128
                return wp[:, c0:c0 + 128]

            def wproj(k):
                return wp[:, 6 * G + k:6 * G + k + 1]

            # persistent state: h history (bf16, feeds matmuls) + c (f32)
            hh = [spool.tile([128, ntot], bf16, tag=f"hh{k}", name=f"hh{k}")
                  for k in range(2)]
            cc = [spool.tile([128, n0], f32, tag=f"cc{k}", name=f"cc{k}")
                  for k in range(2)]

            for s in range(lmax):
                off = int(offs[s])
                poff = int(offs[s - 1]) if s > 0 else 0
                for c0, C in _chunks(npad[s], first_small=(s == 0)):
                    xt = xpool.tile([128, 4, C], bf16, tag="x", name="x")
                    nc.sync.dma_start(xt[:], xg_d[:, :, off + c0:off + c0 + C])
                    for half in range(2):
                        ps = {}
                        for gi, gname in enumerate("ifgo"):
                            if s == 0 and gname == "f":
                                continue
                            gt = gi * 2 + half
                            p = ppool.tile([128, C], f32, tag="ps", name="ps",
                                           bufs=6)
                            for d in range(4):
                                nc.tensor.matmul(
                                    p[:], lhsT=wih(d, gt), rhs=xt[:, d, :],
                                    start=(d == 0), stop=(s == 0 and d == 3))
                            if s > 0:
                                for k in range(2):
                                    nc.tensor.matmul(
                                        p[:], lhsT=whh(k, gt),
                                        rhs=hh[k][:, poff + c0:poff + c0 + C],
                                        start=False, stop=(k == 1))
                            ps[gname] = p

                        def bcol(gi):
                            gt = gi * 2 + half
                            return bias[:, gt:gt + 1]

                        c_sl = cc[half][:, c0:c0 + C]
                        # sigmoids first, then tanhs: fewer ACT table switches
                        sig_i = gpool.tile([128, C], f32, tag="si", name="si")
                        nc.scalar.activation(sig_i[:], ps["i"][:], SIG,
                                             bias=bcol(0))
                        if s > 0:
                            sig_f = gpool.tile([128, C], f32, tag="sf",
                                               name="sf")
                            nc.scalar.activation(sig_f[:], ps["f"][:], SIG,
                                                 bias=bcol(1))
                        sig_o = gpool.tile([128, C], f32, tag="so", name="so")
                        nc.scalar.activation(sig_o[:], ps["o"][:], SIG,
                                             bias=bcol(3))
                        tanh_g = gpool.tile([128, C], f32, tag="tg", name="tg")
                        nc.scalar.activation(tanh_g[:], ps["g"][:], TANH,
                                             bias=bcol(2))
                        if s == 0:
                            nc.vector.tensor_mul(c_sl, sig_i[:], tanh_g[:])
                        else:
                            nc.vector.tensor_mul(c_sl, c_sl, sig_f[:])
                            t1 = gpool.tile([128, C], f32, tag="t1", name="t1")
                            nc.vector.tensor_mul(t1[:], sig_i[:], tanh_g[:])
                            nc.vector.tensor_add(c_sl, c_sl, t1[:])
                        tanh_c = gpool.tile([128, C], f32, tag="tc", name="tc")
                        nc.scalar.activation(tanh_c[:], c_sl, TANH)
                        nc.vector.tensor_mul(hh[half][:, off + c0:off + c0 + C],
                                             sig_o[:], tanh_c[:])

            if export_n:
                lo = int(offs[lmax - 1])
                E = export_n
                for k in range(2):
                    hst = gpool.tile([128, E], f32, tag="hst", name="hst")
                    nc.vector.tensor_copy(hst[:], hh[k][:, lo:lo + E])
                    nc.sync.dma_start(
                        out_d[:, ntot + k * 128 * E:ntot + (k + 1) * 128 * E],
                        hst[:])
                    nc.sync.dma_start(
                        out_d[:, ntot + (2 + k) * 128 * E:
                              ntot + (3 + k) * 128 * E],
                        cc[k][:, 0:E])

            # projection pass: y = W_proj @ h  (b_proj added on host)
            for c0, C in _chunks(ntot):
                p = ppool.tile([1, C], f32, tag="psy", name="psy", bufs=2)
                for k in range(2):
                    nc.tensor.matmul(p[:], lhsT=wproj(k),
                                     rhs=hh[k][:, c0:c0 + C],
                                     start=(k == 0), stop=(k == 1))
                ysb = gpool.tile([1, C], f32, tag="ysb", name="ysb")
                nc.vector.tensor_copy(ysb[:], p[:])
                nc.sync.dma_start(out_d[:, c0:c0 + C], ysb[:])

    nc.compile()
    return nc


def _prep(inputs, eps_per_core, npad, offs, ntot):
    """Build (nc, in_maps) for the SPMD run. npad/offs/ntot are the
    device-side (possibly S_CUT-truncated) schedule."""
    x = np.asarray(inputs["x"], dtype=np.float32)

    # wpack: [wihT (4 d-tiles) | whhT (2 k-tiles) | wprojT (2 cols)] bf16
    wih_t = np.asarray(inputs["W_ih"], np.float32).T  # [D, 4H] = [512, G]
    whh_t = np.asarray(inputs["W_hh"], np.float32).T  # [H, G] = [256, G]
    wproj_t = np.asarray(inputs["W_proj"], np.float32).T  # [H, 1]
    WCOLS = 6 * G + 2
    wpack = np.zeros((128, WCOLS), dtype=np.float32)
    for d in range(4):
        wpack[:, d * G:(d + 1) * G] = wih_t[d * 128:(d + 1) * 128, :]
    for k in range(2):
        wpack[:, 4 * G + k * G:4 * G + (k + 1) * G] = \
            whh_t[k * 128:(k + 1) * 128, :]
    for k in range(2):
        wpack[:, 6 * G + k] = wproj_t[k * 128:(k + 1) * 128, 0]
    wpack = wpack.astype(_BF16)

    bias_flat = (np.asarray(inputs["b_ih"], np.float32)
                 + np.asarray(inputs["b_hh"], np.float32))
    bias_r = np.ascontiguousarray(bias_flat.reshape(8, 128).T)

    x2d = x.reshape(T * B, D)
    in_maps = []
    for c in range(N_CORES):
        gidx = _build_gather(eps_per_core[c], npad, offs, ntot)
        xr = x2d[np.maximum(gidx, 0)]       # [NTOT, D] f32
        xr[gidx < 0] = 0.0
        # xg[p, d, col] = xr[col, d*128+p]
        xg = np.ascontiguousarray(
            xr.T.reshape(4, 128, ntot).transpose(1, 0, 2)).astype(_BF16)
        in_maps.append({"xg": xg, "wpack": wpack, "bias": bias_r})

    export_n = npad[-1] if len(npad) == S_CUT else 0
    nc = _build_bass(npad, offs, ntot, export_n=export_n)
    return nc, in_maps


def _host_tail(out, hexp_cexp, eps_per_core, npad, inputs):
    """Finish episodes longer than S_CUT in exact f32 on the host, starting
    from the device-exported (h, c) state at step S_CUT-1."""
    x2d = np.asarray(inputs["x"], np.float32).reshape(T * B, D)
    W_ih = np.asarray(inputs["W_ih"], np.float32)
    W_hh = np.asarray(inputs["W_hh"], np.float32)
    bvec = (np.asarray(inputs["b_ih"], np.float32)
            + np.asarray(inputs["b_hh"], np.float32))
    W_proj = np.asarray(inputs["W_proj"], np.float32).reshape(-1)
    bp = np.float32(np.asarray(inputs["b_proj"]).reshape(-1)[0])
    s_cut = len(npad)
    sig = lambda v: 1.0 / (1.0 + np.exp(-v))
    for c in range(N_CORES):
        eps = [e for e in eps_per_core[c] if e[2] > s_cut]
        if not eps:
            continue
        n = len(eps)  # eps are ranks 0..n-1 (sorted desc, stable)
        hexp, cexp = hexp_cexp[c]
        h = hexp.reshape(256, -1)[:, :n].T.copy()  # [n, 256]
        cst = cexp.reshape(256, -1)[:, :n].T.copy()
        alive = list(range(n))
        s = s_cut
        while alive:
            keep = [i for i in alive if eps[i][2] > s]
            if not keep:
                break
            rows = np.array([(eps[i][1] + s) * B + eps[i][0] for i in keep])
            idx = np.array(keep)
            g = x2d[rows] @ W_ih.T + h[idx] @ W_hh.T + bvec
            i_, f_, g_, o_ = np.split(g, 4, axis=1)
            cst[idx] = sig(f_) * cst[idx] + sig(i_) * np.tanh(g_)
            hn = sig(o_) * np.tanh(cst[idx])
            h[idx] = hn
            out[rows] = hn @ W_proj + bp
            alive = keep
            s += 1


def _device_schedule(eps_per_core):
    """Full schedule truncated to the device's S_CUT window."""
    npad, offs, ntot = _schedule(eps_per_core)
    if len(npad) > S_CUT:
        npad = npad[:S_CUT]
        offs = offs[:S_CUT + 1]
        ntot = int(offs[-1])
    return npad, offs, ntot


def kernel(x, reset, W_ih, W_hh, b_ih, b_hh, W_proj, b_proj):
    from concourse.bass_utils import run_bass_kernel_spmd

    inputs = dict(x=x, reset=reset, W_ih=W_ih, W_hh=W_hh, b_ih=b_ih,
                  b_hh=b_hh, W_proj=W_proj, b_proj=b_proj)
    reset = np.asarray(reset)
    eps_per_core = _episodes_per_core(reset)
    npad, offs, ntot = _device_schedule(eps_per_core)
    nc, in_maps = _prep(inputs, eps_per_core, npad, offs, ntot)
    res = run_bass_kernel_spmd(nc, in_maps, core_ids=list(range(N_CORES)))

    export_n = npad[-1] if len(npad) == S_CUT else 0
    out = np.empty(T * B, dtype=np.float32)
    bp = np.float32(np.asarray(b_proj).reshape(-1)[0])
    hexp_cexp = []
    for c in range(N_CORES):
        gidx = _build_gather(eps_per_core[c], npad, offs, ntot)
        full = np.asarray(res.results[c]["out"]).reshape(-1)
        y = full[:ntot]
        valid = gidx >= 0
        out[gidx[valid]] = y[valid] + bp
        if export_n:
            E = export_n
            hexp = full[ntot:ntot + 2 * 128 * E].reshape(2, 128, E)
            cexp = full[ntot + 2 * 128 * E:ntot + 4 * 128 * E].reshape(
                2, 128, E)
            hexp_cexp.append((
                np.concatenate([hexp[0], hexp[1]], axis=0),
                np.concatenate([cexp[0], cexp[1]], axis=0)))
    if export_n:
        _host_tail(out, hexp_cexp, eps_per_core, npad, inputs)
    return out.reshape(T, B, 1)
